# revision 6
# baseline (speedup 1.0000x reference)
"""Trainium2 Bass kernel for nn_NMSquaredGaussianMixture.

Math: output = -(log(sum_n g_n^2) - log z) / N
  g_n = sum_k c_k * exp(E_k(x_n)),  c_k = w_k / sqrt((2pi)^2 det S_k)
  E_k(x) = -0.5 (x-mu_k)^T S_k^{-1} (x-mu_k)
  z     = sum_ij w_i w_j N(mu_i - mu_j; 0, S_i + S_j)   (tiny, host-side)

Device pipeline (per core, data-parallel over samples):
  mm1:  E = W1 @ F  in bf16 (1 cyc/row vs fp32r's fp32_mode=HIGH 2 cyc/row).
        F = 8 feature slots [y0^2, y0*y1, y1^2, y0, y0, y1, y1, y0^2] in a
        re-centered basis; the duplicated slots carry hi/lo bf16 splits of
        the linear (and one quadratic) coefficients, and the constant term
        (logc - 0.5 mPm) rides the fp32 activation bias instead of a bf16
        slot -- together this pushes the bf16-weight rounding bias on the
        final scalar from ~4e-3 down to ~1e-4 (measured offline).
  exp:  per half-tile (8 clusters x 16 groups x 512 samples), engine chosen
        round-robin: ACT exp with per-partition fp32 bias, or DVE bf16
        Schraudolph (bits ~= E*128/ln2 + B2[k], negative saturates to +0.0)
        to keep both engines under the PE's supertile period.
  mm2:  sign-combine [128->16, 512] bf16 matmuls, output packed at
        partition offset 16*(st%8) so 8 supertiles fill a [128, 512] PSUM
        tile; then one DVE tensor_tensor_reduce squares + row-reduces it
        into acc[:, block]. Output per core is acc [128, 4] f32 (2KB) --
        no wide writeback, no host-side 2M-element postprocess.

DMA: rhs chunk 0 is issued before the weights so the first matmul's data
is in flight during the fixed ~7us program preamble; warm-up matmuls run
on memset tiles (no DMA dependency) to heat the PE HAM clock-gate while
the first chunks land.
"""

import numpy as np

import concourse.bass as bass
import concourse.mybir as mybir
import concourse.tile as tile
from concourse import bacc
from concourse.bass_utils import run_bass_kernel_spmd

N_SAMPLES = 2_000_000
N_CORES = 8
NC_SAMP = N_SAMPLES // N_CORES  # 250_000
K = 16  # clusters
NF = 8  # feature slots
G = 16  # sample groups (one per matmul output column block)
FD = 512  # moving free dim (one PSUM bank of fp32)
SUPER = G * FD  # samples per super-tile = 8192
NST = 32  # super-tiles/core (multiple of 8 so g-blocks are full)
NPAD = NST * SUPER  # 262144 padded samples per core
NBLK = NST // 8  # g-square blocks
PAD_U = 1.0e6  # pad feature: huge y0^2 --> E << 0 --> dens = 0

TWO_PI = 2.0 * np.pi
# bf16-Schraudolph exp constants: bits(bf16 exp(v)) ~= v*128/ln2 + (127*128-C2)
# C2 = 8.0 calibrated end-to-end (cancels the piecewise-linear bias).
SCHRAUD_A2 = float(128.0 / np.log(2.0))
SCHRAUD_B2 = float(127.0 * 128.0 - 8.0)
# pipeline tuning knobs
RHS_CHUNKS = [1, 1, 2, 4, 6, 6, 6, 6]
RHS_BUFS = 3
DENS_BUFS = 5
E_BUFS = 2
G_BUFS = 2
PIPE_D = 2  # supertile lag between mm1 emission and mm2 consumption
# exp engine per half-tile unit (u = st*2 + half), cycled: A=ACT exp,
# D=DVE Schraudolph.
EXP_PAT = "AAADAADA"
NWARM = 8

_CACHE = {}


def _bf16_np():
    import ml_dtypes

    return ml_dtypes.bfloat16


def _cluster_params(means, chols, weights):
    """Monomial coefficients A [K,6] (f64) of E_k in a re-centered basis
    (A columns: y0^2, y0*y1, y1^2, y0, y1, const incl logc), signs [K],
    center ctr [2]."""
    means = np.asarray(means, np.float64)
    chols = np.asarray(chols, np.float64)
    weights = np.asarray(weights, np.float64)
    L = np.tril(chols)
    S = L @ np.swapaxes(L, 1, 2)
    P = np.linalg.inv(S)
    detS = np.linalg.det(S)
    c = weights / np.sqrt(TWO_PI**2 * detS)
    signs = np.where(c >= 0, 1.0, -1.0)
    logc = np.log(np.abs(c))
    pw = np.abs(P).sum(axis=(1, 2))
    ctr = (means * pw[:, None]).sum(0) / pw.sum()
    m = means - ctr[None, :]
    Pm = np.einsum("kij,kj->ki", P, m)
    mPm = np.einsum("ki,ki->k", m, Pm)
    A = np.stack(
        [
            -0.5 * P[:, 0, 0],
            -P[:, 0, 1],
            -0.5 * P[:, 1, 1],
            Pm[:, 0],
            Pm[:, 1],
            -0.5 * mPm + logc,
        ],
        axis=1,
    )
    return A, signs, ctr


def _z_term(means, chols, weights):
    means = np.asarray(means, np.float64)
    chols = np.asarray(chols, np.float64)
    weights = np.asarray(weights, np.float64)
    L = np.tril(chols)
    S = L @ np.swapaxes(L, 1, 2)
    Ssum = S[:, None] + S[None, :]
    mdiff = means[:, None, :] - means[None, :, :]
    m2 = np.einsum("abi,abij,abj->ab", mdiff, np.linalg.inv(Ssum), mdiff)
    Zij = np.exp(-0.5 * m2) / np.sqrt(TWO_PI**2 * np.linalg.det(Ssum))
    return float(np.einsum("i,j,ij->", weights, weights, Zij))


def _build_rhs(X, ctr):
    """X [2M,2] f32 -> per-core rhs [N_CORES, 128, NST*FD] bf16, where
    rhs[c, s*G+g, st*FD + t] = feat_s of sample n = c*NC_SAMP + st*SUPER
    + g*FD + t (pad samples give dens == 0). Slots s: [q0 q01 q1 x0 x0 x1
    x1 q0]."""
    bf16 = _bf16_np()
    X = np.asarray(X, np.float32)
    feats = np.zeros((N_CORES, NPAD, NF), np.float32)
    x0 = (X[:, 0] - np.float32(ctr[0])).reshape(N_CORES, NC_SAMP)
    x1 = (X[:, 1] - np.float32(ctr[1])).reshape(N_CORES, NC_SAMP)
    q0 = x0 * x0
    feats[:, :NC_SAMP, 0] = q0
    feats[:, :NC_SAMP, 1] = x0 * x1
    feats[:, :NC_SAMP, 2] = x1 * x1
    feats[:, :NC_SAMP, 3] = x0
    feats[:, :NC_SAMP, 4] = x0
    feats[:, :NC_SAMP, 5] = x1
    feats[:, :NC_SAMP, 6] = x1
    feats[:, :NC_SAMP, 7] = q0
    feats[:, NC_SAMP:, 0] = PAD_U
    feats[:, NC_SAMP:, 7] = PAD_U
    # [C, NST, G, FD, NF] -> [C, NF, G, NST, FD] -> [C, 128, NST*FD]
    r = feats.reshape(N_CORES, NST, G, FD, NF).transpose(0, 4, 2, 1, 3)
    return np.ascontiguousarray(r).reshape(N_CORES, NF * G, NST * FD).astype(bf16)


def _build_weights(A, signs):
    """w1 [128, 256] bf16 block coefficient mats (cluster halves);
    cm [128, 32] bf16 sign-combine mats; cv [128, 4] f32 per-partition
    constants (exp bias h0/h1, Schraudolph B2 h0/h1).

    Out partition m = kl*G + g (kl = cluster within half). Contraction
    partition p = s*G + g. Slot weights: [hi(A0), A1, A2, hi(A3), lo(A3),
    hi(A4), lo(A4), lo(A0)]; const A5 rides the activation bias."""
    bf16 = _bf16_np()

    def hi_lo(v):
        h = v.astype(bf16).astype(np.float64)
        l = (v - h).astype(bf16).astype(np.float64)
        return h, l

    a0h, a0l = hi_lo(A[:, 0])
    a3h, a3l = hi_lo(A[:, 3])
    a4h, a4l = hi_lo(A[:, 4])
    slotw = np.stack(
        [a0h, A[:, 1], A[:, 2], a3h, a3l, a4h, a4l, a0l], axis=1
    )  # [K, 8]
    const = A[:, 5]

    w1 = np.zeros((NF * G, 2 * 128), np.float32)
    # cm: 8 column-blocks of 64, one per (variant v = st%4, half h). Block
    # (v*2+h) holds the half-h signs in columns v*16..v*16+16 and zeros
    # elsewhere, so four super-tiles accumulate into disjoint 16-row
    # stripes of one 64-row PSUM slab (matmul out base must be 0/32/64).
    cm = np.zeros((128, 8 * 64), np.float32)
    cv = np.zeros((128, 4), np.float32)
    for half in (0, 1):
        for kl in range(8):
            k = half * 8 + kl
            for g in range(G):
                m = kl * G + g
                for s in range(NF):
                    w1[s * G + g, half * 128 + m] = slotw[k, s]
                for v in range(4):
                    cm[m, (v * 2 + half) * 64 + v * 16 + g] = signs[k]
                cv[m, half] = const[k]
                cv[m, 2 + half] = const[k] * SCHRAUD_A2 + SCHRAUD_B2
    return w1.astype(bf16), cm.astype(bf16), cv


def _build_bass():
    nc = bacc.Bacc("TRN2", target_bir_lowering=False, debug=False)
    f32 = mybir.dt.float32
    bf16 = mybir.dt.bfloat16
    u16 = mybir.dt.uint16
    rhs_d = nc.dram_tensor("rhs", [NF * G, NST * FD], bf16, kind="ExternalInput")
    w1_d = nc.dram_tensor("w1", [NF * G, 2 * 128], bf16, kind="ExternalInput")
    cm_d = nc.dram_tensor("cm", [128, 8 * 64], bf16, kind="ExternalInput")
    cv_d = nc.dram_tensor("cv", [128, 4], f32, kind="ExternalInput")
    acc_d = nc.dram_tensor("acc", [128, NBLK], f32, kind="ExternalOutput")

    assert sum(RHS_CHUNKS) == NST

    with tile.TileContext(nc) as tc:
        with (
            tc.tile_pool(name="const", bufs=1) as cpool,
            tc.tile_pool(name="rhs", bufs=RHS_BUFS) as rpool,
            tc.tile_pool(name="dens", bufs=DENS_BUFS) as dpool,
            tc.tile_pool(name="pe", bufs=E_BUFS, space="PSUM") as epool,
            tc.tile_pool(name="pg", bufs=G_BUFS, space="PSUM") as gpool,
            tc.tile_pool(name="pw", bufs=1, space="PSUM") as wpool,
        ):
            w1 = cpool.tile([NF * G, 2 * 128], bf16)
            cm = cpool.tile([128, 8 * 64], bf16)
            cv = cpool.tile([128, 4], f32)
            acc = cpool.tile([128, NBLK], f32)
            scr = cpool.tile([128, FD], bf16)  # square dummy out
            wsb = cpool.tile([128, FD], bf16)  # warm-up operand

            # rhs chunk 0 first: its transfer overlaps the remaining issue
            # chain and the program preamble.
            rhs_views = {}
            chunk_tiles = []
            lo = 0
            for ci, sz in enumerate(RHS_CHUNKS):
                hi = lo + sz
                rt = rpool.tile([NF * G, max(RHS_CHUNKS) * FD], bf16, tag="rhs")
                nc.sync.dma_start(rt[:, : sz * FD], rhs_d[:, lo * FD : hi * FD])
                for st in range(lo, hi):
                    rhs_views[st] = rt[:, (st - lo) * FD : (st - lo + 1) * FD]
                lo = hi
                chunk_tiles.append(rt)
                if ci == 0:
                    nc.sync.dma_start(w1[:], w1_d[:])
                    nc.sync.dma_start(cv[:], cv_d[:])
                    nc.sync.dma_start(cm[:], cm_d[:])

            # PE warm-up on memset tiles (no DMA dependency): heats the HAM
            # clock-gate during the preamble + first chunk's flight time.
            nc.vector.memset(wsb[:], 0.0)
            warm = wpool.tile([128, FD], f32)
            for _ in range(NWARM):
                nc.tensor.matmul(
                    warm[0:128, :], wsb[:, 0:128], wsb[:], start=True, stop=True
                )

            dens_ring = [None] * NST
            g_ring = [None, None]

            def emit_front(st):
                rhs = rhs_views[st]
                e = epool.tile([128, 2 * FD], f32, tag="e")
                d = dpool.tile([128, 2 * FD], u16, tag="dens")
                nc.tensor.matmul(
                    e[:, 0:FD], w1[:, 0:128], rhs, start=True, stop=True
                )
                nc.tensor.matmul(
                    e[:, FD : 2 * FD], w1[:, 128:256], rhs, start=True, stop=True
                )
                for h in (0, 1):
                    eng = EXP_PAT[(st * 2 + h) % len(EXP_PAT)]
                    eh = e[:, h * FD : (h + 1) * FD]
                    dh = d[:, h * FD : (h + 1) * FD]
                    if eng == "A":
                        nc.scalar.activation(
                            dh.bitcast(bf16),
                            eh,
                            mybir.ActivationFunctionType.Exp,
                            bias=cv[:, h : h + 1],
                        )
                    else:
                        nc.vector.tensor_scalar(
                            dh,
                            eh,
                            SCHRAUD_A2,
                            cv[:, 2 + h : 3 + h],
                            op0=mybir.AluOpType.mult,
                            op1=mybir.AluOpType.add,
                        )
                dens_ring[st] = d

            def emit_back(st):
                d = dens_ring[st]
                dens_ring[st] = None
                j = st % 8
                slab, v = j // 4, j % 4
                if j == 0:
                    g_ring[(st // 8) % 2] = gpool.tile(
                        [128, FD], f32, tag="g", name=f"g128_{st // 8}"
                    )
                g128 = g_ring[(st // 8) % 2]
                dbf = d[:].bitcast(bf16)
                out = g128[slab * 64 : (slab + 1) * 64, :]
                for h in (0, 1):
                    blkcol = (v * 2 + h) * 64
                    nc.tensor.matmul(
                        out,
                        cm[:, blkcol : blkcol + 64],
                        dbf[:, h * FD : (h + 1) * FD],
                        start=(v == 0 and h == 0),
                        stop=(v == 3 and h == 1),
                        skip_group_check=True,
                    )
                if j == 7:
                    blk = st // 8
                    nc.scalar.activation(
                        scr[:],
                        g128[:],
                        mybir.ActivationFunctionType.Square,
                        accum_out=acc[:, blk : blk + 1],
                    )

            D = PIPE_D
            for t in range(NST + D):
                if t < NST:
                    emit_front(t)
                if t >= D:
                    emit_back(t - D)

            nc.sync.dma_start(acc_d[:], acc[:])

    nc.compile()
    return nc


def _get_bass():
    if "nc" not in _CACHE:
        _CACHE["nc"] = _build_bass()
    return _CACHE["nc"]


def kernel(X, means, chols, weights, it=None, **_unused):
    X = np.ascontiguousarray(np.asarray(X, np.float32))
    assert X.shape == (N_SAMPLES, 2), X.shape

    A, signs, ctr = _cluster_params(means, chols, weights)
    z = _z_term(means, chols, weights)
    w1, cm, cv = _build_weights(A, signs)
    rhs = _build_rhs(X, ctr)

    nc = _get_bass()
    in_maps = [
        {"rhs": rhs[c], "w1": w1, "cm": cm, "cv": cv} for c in range(N_CORES)
    ]
    res = run_bass_kernel_spmd(nc, in_maps, core_ids=list(range(N_CORES)))

    total = 0.0
    for r in res.results:
        total += float(r["acc"].astype(np.float64).sum())

    out = -(np.log(total) - np.log(z)) / N_SAMPLES
    return np.float32(out)


if __name__ == "__main__":
    rng = np.random.default_rng(0)
    X = rng.standard_normal((N_SAMPLES, 2), dtype=np.float32)
    scale = 2.0 * (1.0 + rng.standard_normal((K, 1, 1), dtype=np.float32))
    chols = scale * np.ones((2, 2), np.float32)[None] + 0.5 * np.eye(2, dtype=np.float32)[None]
    means = rng.standard_normal((K, 2), dtype=np.float32)
    weights = rng.standard_normal(K, dtype=np.float32)
    print(kernel(X, means, chols, weights, 1))


# revision 8
# speedup vs baseline: 1.3568x; 1.3568x over previous
"""Trainium2 Bass kernel for nn_NMSquaredGaussianMixture.

Math: output = -(log(sum_n g_n^2) - log z) / N
  g_n = sum_k c_k * exp(E_k(x_n)),  c_k = w_k / sqrt((2pi)^2 det S_k)
  E_k(x) = -0.5 (x-mu_k)^T S_k^{-1} (x-mu_k)
  z     = sum_ij w_i w_j N(mu_i - mu_j; 0, S_i + S_j)   (tiny, host-side)

Device pipeline (per core, data-parallel over samples):
  mm1:  E = W1 @ F  in bf16 (1 cyc/row vs fp32r's fp32_mode=HIGH 2 cyc/row).
        F = 8 feature slots [y0^2, y0*y1, y1^2, y0, y0, y1, y1, y0^2] in a
        re-centered basis; the duplicated slots carry hi/lo bf16 splits of
        the linear (and one quadratic) coefficients, and the constant term
        (logc - 0.5 mPm) rides the fp32 activation bias instead of a bf16
        slot -- together this pushes the bf16-weight rounding bias on the
        final scalar from ~4e-3 down to ~1e-4 (measured offline).
  exp:  per half-tile (8 clusters x 16 groups x 512 samples), engine chosen
        round-robin: ACT exp with per-partition fp32 bias, or DVE bf16
        Schraudolph (bits ~= E*128/ln2 + B2[k], negative saturates to +0.0)
        to keep both engines under the PE's supertile period.
  mm2:  sign-combine [128->16, 512] bf16 matmuls, output packed at
        partition offset 16*(st%8) so 8 supertiles fill a [128, 512] PSUM
        tile; then one DVE tensor_tensor_reduce squares + row-reduces it
        into acc[:, block]. Output per core is acc [128, 4] f32 (2KB) --
        no wide writeback, no host-side 2M-element postprocess.

DMA: rhs chunk 0 is issued before the weights so the first matmul's data
is in flight during the fixed ~7us program preamble; warm-up matmuls run
on memset tiles (no DMA dependency) to heat the PE HAM clock-gate while
the first chunks land.
"""

import numpy as np

import concourse.bass as bass
import concourse.mybir as mybir
import concourse.tile as tile
from concourse import bacc
from concourse.bass_utils import run_bass_kernel_spmd

N_SAMPLES = 2_000_000
N_CORES = 8
NC_SAMP = N_SAMPLES // N_CORES  # 250_000
K = 16  # clusters
NF = 8  # feature slots
G = 16  # sample groups (one per matmul output column block)
FD = 512  # moving free dim (one PSUM bank of fp32)
SUPER = G * FD  # samples per super-tile = 8192
NST = 32  # super-tiles/core (multiple of 8 so g-blocks are full)
NPAD = NST * SUPER  # 262144 padded samples per core
NBLK = NST // 8  # g-square blocks
PAD_U = 1.0e6  # pad feature: huge y0^2 --> E << 0 --> dens = 0

TWO_PI = 2.0 * np.pi
# bf16-Schraudolph exp constants: bits(bf16 exp(v)) ~= v*128/ln2 + (127*128-C2)
# C2 = 8.0 calibrated end-to-end (cancels the piecewise-linear bias).
SCHRAUD_A1 = float(8.0 / np.log(2.0))
SCHRAUD_B1 = float(7.0 * 8.0 - 0.0)
# pipeline tuning knobs
RHS_CHUNKS = [1, 1, 2, 4, 6, 6, 6, 6]
RHS_BUFS = 3
DENS_BUFS = 5
E_BUFS = 3
G_BUFS = 2
PIPE_D = 2  # supertile lag between mm1 emission and mm2 consumption
# exp engine per half-tile unit (u = st*2 + half), cycled: A=ACT exp,
# D=DVE fp8 Schraudolph.
EXP_PAT = "AADADAAD"
NWARM = 10
WARM_FD = 128

_CACHE = {}


def _bf16_np():
    import ml_dtypes

    return ml_dtypes.bfloat16


def _cluster_params(means, chols, weights):
    """Monomial coefficients A [K,6] (f64) of E_k in a re-centered basis
    (A columns: y0^2, y0*y1, y1^2, y0, y1, const incl logc), signs [K],
    center ctr [2]."""
    means = np.asarray(means, np.float64)
    chols = np.asarray(chols, np.float64)
    weights = np.asarray(weights, np.float64)
    L = np.tril(chols)
    S = L @ np.swapaxes(L, 1, 2)
    P = np.linalg.inv(S)
    detS = np.linalg.det(S)
    c = weights / np.sqrt(TWO_PI**2 * detS)
    signs = np.where(c >= 0, 1.0, -1.0)
    logc = np.log(np.abs(c))
    pw = np.abs(P).sum(axis=(1, 2))
    ctr = (means * pw[:, None]).sum(0) / pw.sum()
    m = means - ctr[None, :]
    Pm = np.einsum("kij,kj->ki", P, m)
    mPm = np.einsum("ki,ki->k", m, Pm)
    A = np.stack(
        [
            -0.5 * P[:, 0, 0],
            -P[:, 0, 1],
            -0.5 * P[:, 1, 1],
            Pm[:, 0],
            Pm[:, 1],
            -0.5 * mPm + logc,
        ],
        axis=1,
    )
    return A, signs, ctr


def _z_term(means, chols, weights):
    means = np.asarray(means, np.float64)
    chols = np.asarray(chols, np.float64)
    weights = np.asarray(weights, np.float64)
    L = np.tril(chols)
    S = L @ np.swapaxes(L, 1, 2)
    Ssum = S[:, None] + S[None, :]
    mdiff = means[:, None, :] - means[None, :, :]
    m2 = np.einsum("abi,abij,abj->ab", mdiff, np.linalg.inv(Ssum), mdiff)
    Zij = np.exp(-0.5 * m2) / np.sqrt(TWO_PI**2 * np.linalg.det(Ssum))
    return float(np.einsum("i,j,ij->", weights, weights, Zij))


def _build_rhs(X, ctr):
    """X [2M,2] f32 -> per-core rhs [N_CORES, 128, NST*FD] bf16, where
    rhs[c, s*G+g, st*FD + t] = feat_s of sample n = c*NC_SAMP + st*SUPER
    + g*FD + t (pad samples give dens == 0). Slots s: [q0 q01 q1 x0 x0 x1
    x1 q0]."""
    bf16 = _bf16_np()
    X = np.asarray(X, np.float32)
    feats = np.zeros((N_CORES, NPAD, NF), np.float32)
    x0 = (X[:, 0] - np.float32(ctr[0])).reshape(N_CORES, NC_SAMP)
    x1 = (X[:, 1] - np.float32(ctr[1])).reshape(N_CORES, NC_SAMP)
    q0 = x0 * x0
    feats[:, :NC_SAMP, 0] = q0
    feats[:, :NC_SAMP, 1] = x0 * x1
    feats[:, :NC_SAMP, 2] = x1 * x1
    feats[:, :NC_SAMP, 3] = x0
    feats[:, :NC_SAMP, 4] = x0
    feats[:, :NC_SAMP, 5] = x1
    feats[:, :NC_SAMP, 6] = x1
    feats[:, :NC_SAMP, 7] = q0
    feats[:, NC_SAMP:, 0] = PAD_U
    feats[:, NC_SAMP:, 7] = PAD_U
    # [C, NST, G, FD, NF] -> [C, NF, G, NST, FD] -> [C, 128, NST*FD]
    r = feats.reshape(N_CORES, NST, G, FD, NF).transpose(0, 4, 2, 1, 3)
    return np.ascontiguousarray(r).reshape(N_CORES, NF * G, NST * FD).astype(bf16)


def _build_weights(A, signs):
    """w1 [128, 256] bf16 block coefficient mats (cluster halves);
    cm [128, 32] bf16 sign-combine mats; cv [128, 4] f32 per-partition
    constants (exp bias h0/h1, Schraudolph B2 h0/h1).

    Out partition m = kl*G + g (kl = cluster within half). Contraction
    partition p = s*G + g. Slot weights: [hi(A0), A1, A2, hi(A3), lo(A3),
    hi(A4), lo(A4), lo(A0)]; const A5 rides the activation bias."""
    bf16 = _bf16_np()

    def hi_lo(v):
        h = v.astype(bf16).astype(np.float64)
        l = (v - h).astype(bf16).astype(np.float64)
        return h, l

    a0h, a0l = hi_lo(A[:, 0])
    a3h, a3l = hi_lo(A[:, 3])
    a4h, a4l = hi_lo(A[:, 4])
    slotw = np.stack(
        [a0h, A[:, 1], A[:, 2], a3h, a3l, a4h, a4l, a0l], axis=1
    )  # [K, 8]
    const = A[:, 5]

    import ml_dtypes

    f8 = ml_dtypes.float8_e4m3fn
    logcmax = float(const.max())  # E + const <= const <= logcmax => dens <= 1
    w1 = np.zeros((NF * G, 2 * 128), np.float32)
    # cm: DoubleRow lhsT [128, 2*8, 128]: for variant v = st%8 the matmul
    # uses subtile pair (2v, 2v+1) = (half0, half1) signs, nonzero only in
    # out rows v*16..v*16+16 -- eight super-tiles accumulate into disjoint
    # 16-row stripes of one full [128, FD] PSUM tile (DoubleRow dst
    # partition base must be 0, so the out always spans all 128 rows).
    cm = np.zeros((128, 16, 128), np.float32)
    cv = np.zeros((128, 4), np.float32)
    for half in (0, 1):
        for kl in range(8):
            k = half * 8 + kl
            for g in range(G):
                m = kl * G + g
                for s in range(NF):
                    w1[s * G + g, half * 128 + m] = slotw[k, s]
                for v in range(8):
                    cm[m, v * 2 + half, v * 16 + g] = signs[k]
                cv[m, half] = const[k] - logcmax
                cv[m, 2 + half] = (const[k] - logcmax) * SCHRAUD_A1 + SCHRAUD_B1
    return w1.astype(bf16), cm.reshape(128, 16 * 128).astype(f8), cv, logcmax


def _build_bass():
    nc = bacc.Bacc("TRN2", target_bir_lowering=False, debug=False)
    f32 = mybir.dt.float32
    bf16 = mybir.dt.bfloat16
    f8 = mybir.dt.float8e4
    u8 = mybir.dt.uint8
    rhs_d = nc.dram_tensor("rhs", [NF * G, NST * FD], bf16, kind="ExternalInput")
    w1_d = nc.dram_tensor("w1", [NF * G, 2 * 128], bf16, kind="ExternalInput")
    cm_d = nc.dram_tensor("cm", [128, 16 * 128], f8, kind="ExternalInput")
    cv_d = nc.dram_tensor("cv", [128, 4], f32, kind="ExternalInput")
    acc_d = nc.dram_tensor("acc", [128, NBLK], f32, kind="ExternalOutput")

    assert sum(RHS_CHUNKS) == NST

    with tile.TileContext(nc) as tc:
        with (
            tc.tile_pool(name="const", bufs=1) as cpool,
            tc.tile_pool(name="rhs", bufs=RHS_BUFS) as rpool,
            tc.tile_pool(name="dens", bufs=DENS_BUFS) as dpool,
            tc.tile_pool(name="pe", bufs=E_BUFS, space="PSUM") as epool,
            tc.tile_pool(name="pg", bufs=G_BUFS, space="PSUM") as gpool,
        ):
            w1 = cpool.tile([NF * G, 2 * 128], bf16)
            cm = cpool.tile([128, 16, 128], f8)
            cv = cpool.tile([128, 4], f32)
            acc = cpool.tile([128, NBLK], f32)
            scr = cpool.tile([128, FD], bf16)  # square dummy out
            wsb = cpool.tile([128, FD], bf16)  # warm-up operand

            # rhs chunk 0 first: its transfer overlaps the remaining issue
            # chain and the program preamble.
            rhs_views = {}
            chunk_tiles = []
            lo = 0
            for ci, sz in enumerate(RHS_CHUNKS):
                hi = lo + sz
                rt = rpool.tile([NF * G, max(RHS_CHUNKS) * FD], bf16, tag="rhs")
                nc.sync.dma_start(rt[:, : sz * FD], rhs_d[:, lo * FD : hi * FD])
                for st in range(lo, hi):
                    rhs_views[st] = rt[:, (st - lo) * FD : (st - lo + 1) * FD]
                lo = hi
                chunk_tiles.append(rt)
                if ci == 0:
                    nc.sync.dma_start(w1[:], w1_d[:])
                    nc.sync.dma_start(cv[:], cv_d[:])
                    nc.sync.dma_start(cm[:, :, :], cm_d[:])

            # PE warm-up on memset tiles (no DMA dependency): heats the HAM
            # clock-gate during the preamble + first chunk's flight time.
            nc.vector.memset(wsb[:], 0.0)
            warm = gpool.tile([128, FD], f32, tag="g", name="warm")
            for _ in range(NWARM):
                nc.tensor.matmul(
                    warm[0:128, 0:WARM_FD],
                    wsb[:, 0:128],
                    wsb[:, 0:WARM_FD],
                    start=True,
                    stop=True,
                )

            dens_ring = [None] * NST
            g_ring = [None, None]

            def emit_front(st):
                rhs = rhs_views[st]
                e = epool.tile([128, 2 * FD], f32, tag="e")
                d = dpool.tile([128, 2, FD], f8, tag="dens")
                nc.tensor.matmul(
                    e[:, 0:FD], w1[:, 0:128], rhs, start=True, stop=True
                )
                nc.tensor.matmul(
                    e[:, FD : 2 * FD], w1[:, 128:256], rhs, start=True, stop=True
                )
                for h in (0, 1):
                    eng = EXP_PAT[(st * 2 + h) % len(EXP_PAT)]
                    eh = e[:, h * FD : (h + 1) * FD]
                    dh = d[:, h, :]
                    if eng == "A":
                        nc.scalar.activation(
                            dh,
                            eh,
                            mybir.ActivationFunctionType.Exp,
                            bias=cv[:, h : h + 1],
                        )
                    else:
                        nc.vector.tensor_scalar(
                            dh.bitcast(u8),
                            eh,
                            SCHRAUD_A1,
                            cv[:, 2 + h : 3 + h],
                            op0=mybir.AluOpType.mult,
                            op1=mybir.AluOpType.add,
                        )
                dens_ring[st] = d

            def emit_back(st):
                d = dens_ring[st]
                dens_ring[st] = None
                v = st % 8
                if v == 0:
                    g_ring[(st // 8) % 2] = gpool.tile(
                        [128, FD], f32, tag="g", name=f"g128_{st // 8}"
                    )
                g128 = g_ring[(st // 8) % 2]
                nc.tensor.matmul(
                    g128[:, :],
                    cm[:, 2 * v : 2 * v + 2, :],
                    d[:, :, :],
                    start=(v == 0),
                    stop=(v == 7),
                    perf_mode=mybir.MatmulPerfMode.DoubleRow,
                    skip_group_check=True,
                )
                if v == 7:
                    blk = st // 8
                    nc.scalar.activation(
                        scr[:],
                        g128[:],
                        mybir.ActivationFunctionType.Square,
                        accum_out=acc[:, blk : blk + 1],
                    )

            D = PIPE_D
            for t in range(NST + D):
                if t < NST:
                    emit_front(t)
                if t >= D:
                    emit_back(t - D)

            nc.sync.dma_start(acc_d[:], acc[:])

    nc.compile()
    return nc


def _get_bass():
    if "nc" not in _CACHE:
        _CACHE["nc"] = _build_bass()
    return _CACHE["nc"]


def kernel(X, means, chols, weights, it=None, **_unused):
    X = np.ascontiguousarray(np.asarray(X, np.float32))
    assert X.shape == (N_SAMPLES, 2), X.shape

    A, signs, ctr = _cluster_params(means, chols, weights)
    z = _z_term(means, chols, weights)
    w1, cm, cv, logcmax = _build_weights(A, signs)
    rhs = _build_rhs(X, ctr)

    nc = _get_bass()
    in_maps = [
        {"rhs": rhs[c], "w1": w1, "cm": cm, "cv": cv} for c in range(N_CORES)
    ]
    res = run_bass_kernel_spmd(nc, in_maps, core_ids=list(range(N_CORES)))

    total = 0.0
    for r in res.results:
        total += float(r["acc"].astype(np.float64).sum())
    # dens were scaled by exp(-logcmax) on device; undo on the squared sum.
    out = -(np.log(total) + 2.0 * logcmax - np.log(z)) / N_SAMPLES
    return np.float32(out)


if __name__ == "__main__":
    rng = np.random.default_rng(0)
    X = rng.standard_normal((N_SAMPLES, 2), dtype=np.float32)
    scale = 2.0 * (1.0 + rng.standard_normal((K, 1, 1), dtype=np.float32))
    chols = scale * np.ones((2, 2), np.float32)[None] + 0.5 * np.eye(2, dtype=np.float32)[None]
    means = rng.standard_normal((K, 2), dtype=np.float32)
    weights = rng.standard_normal(K, dtype=np.float32)
    print(kernel(X, means, chols, weights, 1))


# revision 9
# speedup vs baseline: 1.5010x; 1.1063x over previous
"""Trainium2 Bass kernel for nn_NMSquaredGaussianMixture.

Math: output = -(log(sum_n g_n^2) - log z) / N
  g_n = sum_k c_k * exp(E_k(x_n)),  c_k = w_k / sqrt((2pi)^2 det S_k)
  E_k(x) = -0.5 (x-mu_k)^T S_k^{-1} (x-mu_k)
  z     = sum_ij w_i w_j N(mu_i - mu_j; 0, S_i + S_j)   (tiny, host-side)

Device pipeline (per core, data-parallel over samples):
  mm1:  E = W1 @ F  in bf16 (1 cyc/row vs fp32r's fp32_mode=HIGH 2 cyc/row).
        F = 8 feature slots [y0^2, y0*y1, y1^2, y0, y0, y1, y1, y0^2] in a
        re-centered basis; the duplicated slots carry hi/lo bf16 splits of
        the linear (and one quadratic) coefficients, and the constant term
        (logc - 0.5 mPm) rides the fp32 activation bias instead of a bf16
        slot -- together this pushes the bf16-weight rounding bias on the
        final scalar from ~4e-3 down to ~1e-4 (measured offline).
  exp:  per half-tile (8 clusters x 16 groups x 512 samples), engine chosen
        round-robin: ACT exp with per-partition fp32 bias, or DVE bf16
        Schraudolph (bits ~= E*128/ln2 + B2[k], negative saturates to +0.0)
        to keep both engines under the PE's supertile period.
  mm2:  sign-combine [128->16, 512] bf16 matmuls, output packed at
        partition offset 16*(st%8) so 8 supertiles fill a [128, 512] PSUM
        tile; then one DVE tensor_tensor_reduce squares + row-reduces it
        into acc[:, block]. Output per core is acc [128, 4] f32 (2KB) --
        no wide writeback, no host-side 2M-element postprocess.

DMA: rhs chunk 0 is issued before the weights so the first matmul's data
is in flight during the fixed ~7us program preamble; warm-up matmuls run
on memset tiles (no DMA dependency) to heat the PE HAM clock-gate while
the first chunks land.
"""

import numpy as np

import concourse.bass as bass
import concourse.mybir as mybir
import concourse.tile as tile
from concourse import bacc
from concourse.bass_utils import run_bass_kernel_spmd

N_SAMPLES = 2_000_000
N_CORES = 8
NC_SAMP = N_SAMPLES // N_CORES  # 250_000
K = 16  # clusters
NF = 8  # feature slots
G = 16  # sample groups (one per matmul output column block)
FD = 512  # moving free dim (one PSUM bank of fp32)
SUPER = G * FD  # samples per super-tile = 8192
NST = 32  # super-tiles/core (multiple of 8 so g-blocks are full)
NPAD = NST * SUPER  # 262144 padded samples per core
NBLK = NST // 8  # g-square blocks
PAD_U = 1.0e6  # pad feature: huge y0^2 --> E << 0 --> dens = 0

TWO_PI = 2.0 * np.pi
# bf16-Schraudolph exp constants: bits(bf16 exp(v)) ~= v*128/ln2 + (127*128-C2)
# C2 = 8.0 calibrated end-to-end (cancels the piecewise-linear bias).
SCHRAUD_A1 = float(8.0 / np.log(2.0))
SCHRAUD_B1 = float(7.0 * 8.0 - 0.0)
# pipeline tuning knobs
RHS_CHUNKS = [2, 2, 4, 6, 6, 6, 6]
RHS_BUFS = 3
DENS_BUFS = 5
E_BUFS = 3
G_BUFS = 2
PIPE_D = 2  # supertile lag between mm1 emission and mm2 consumption
# exp engine per half-tile unit (u = st*2 + half), cycled: A=ACT exp,
# D=DVE fp8 Schraudolph.
EXP_PAT = "ADADADAD"
NWARM = 4
WARM_FD = 512

_CACHE = {}


def _bf16_np():
    import ml_dtypes

    return ml_dtypes.bfloat16


def _cluster_params(means, chols, weights):
    """Monomial coefficients A [K,6] (f64) of E_k in a re-centered basis
    (A columns: y0^2, y0*y1, y1^2, y0, y1, const incl logc), signs [K],
    center ctr [2]."""
    means = np.asarray(means, np.float64)
    chols = np.asarray(chols, np.float64)
    weights = np.asarray(weights, np.float64)
    L = np.tril(chols)
    S = L @ np.swapaxes(L, 1, 2)
    P = np.linalg.inv(S)
    detS = np.linalg.det(S)
    c = weights / np.sqrt(TWO_PI**2 * detS)
    signs = np.where(c >= 0, 1.0, -1.0)
    logc = np.log(np.abs(c))
    pw = np.abs(P).sum(axis=(1, 2))
    ctr = (means * pw[:, None]).sum(0) / pw.sum()
    m = means - ctr[None, :]
    Pm = np.einsum("kij,kj->ki", P, m)
    mPm = np.einsum("ki,ki->k", m, Pm)
    A = np.stack(
        [
            -0.5 * P[:, 0, 0],
            -P[:, 0, 1],
            -0.5 * P[:, 1, 1],
            Pm[:, 0],
            Pm[:, 1],
            -0.5 * mPm + logc,
        ],
        axis=1,
    )
    return A, signs, ctr


def _z_term(means, chols, weights):
    means = np.asarray(means, np.float64)
    chols = np.asarray(chols, np.float64)
    weights = np.asarray(weights, np.float64)
    L = np.tril(chols)
    S = L @ np.swapaxes(L, 1, 2)
    Ssum = S[:, None] + S[None, :]
    mdiff = means[:, None, :] - means[None, :, :]
    m2 = np.einsum("abi,abij,abj->ab", mdiff, np.linalg.inv(Ssum), mdiff)
    Zij = np.exp(-0.5 * m2) / np.sqrt(TWO_PI**2 * np.linalg.det(Ssum))
    return float(np.einsum("i,j,ij->", weights, weights, Zij))


def _build_rhs(X, ctr):
    """X [2M,2] f32 -> per-core rhs [N_CORES, 128, NST*FD] bf16, where
    rhs[c, s*G+g, st*FD + t] = feat_s of sample n = c*NC_SAMP + st*SUPER
    + g*FD + t (pad samples give dens == 0). Slots s: [q0 q01 q1 x0 x0 x1
    x1 q0]."""
    bf16 = _bf16_np()
    X = np.asarray(X, np.float32)
    feats = np.zeros((N_CORES, NPAD, NF), np.float32)
    x0 = (X[:, 0] - np.float32(ctr[0])).reshape(N_CORES, NC_SAMP)
    x1 = (X[:, 1] - np.float32(ctr[1])).reshape(N_CORES, NC_SAMP)
    q0 = x0 * x0
    feats[:, :NC_SAMP, 0] = q0
    feats[:, :NC_SAMP, 1] = x0 * x1
    feats[:, :NC_SAMP, 2] = x1 * x1
    feats[:, :NC_SAMP, 3] = x0
    feats[:, :NC_SAMP, 4] = x0
    feats[:, :NC_SAMP, 5] = x1
    feats[:, :NC_SAMP, 6] = x1
    feats[:, :NC_SAMP, 7] = q0
    feats[:, NC_SAMP:, 0] = PAD_U
    feats[:, NC_SAMP:, 7] = PAD_U
    # [C, NST, G, FD, NF] -> [C, NF, G, NST, FD] -> [C, 128, NST*FD]
    r = feats.reshape(N_CORES, NST, G, FD, NF).transpose(0, 4, 2, 1, 3)
    return np.ascontiguousarray(r).reshape(N_CORES, NF * G, NST * FD).astype(bf16)


def _build_weights(A, signs):
    """w1 [128, 256] bf16 block coefficient mats (cluster halves);
    cm [128, 32] bf16 sign-combine mats; cv [128, 4] f32 per-partition
    constants (exp bias h0/h1, Schraudolph B2 h0/h1).

    Out partition m = kl*G + g (kl = cluster within half). Contraction
    partition p = s*G + g. Slot weights: [hi(A0), A1, A2, hi(A3), lo(A3),
    hi(A4), lo(A4), lo(A0)]; const A5 rides the activation bias."""
    bf16 = _bf16_np()

    def hi_lo(v):
        h = v.astype(bf16).astype(np.float64)
        l = (v - h).astype(bf16).astype(np.float64)
        return h, l

    a0h, a0l = hi_lo(A[:, 0])
    a3h, a3l = hi_lo(A[:, 3])
    a4h, a4l = hi_lo(A[:, 4])
    slotw = np.stack(
        [a0h, A[:, 1], A[:, 2], a3h, a3l, a4h, a4l, a0l], axis=1
    )  # [K, 8]
    const = A[:, 5]

    import ml_dtypes

    f8 = ml_dtypes.float8_e4m3fn
    logcmax = float(const.max())  # E + const <= const <= logcmax => dens <= 1
    w1 = np.zeros((NF * G, 2 * 128), np.float32)
    # cm: DoubleRow lhsT [128, 2*8, 128]: for variant v = st%8 the matmul
    # uses subtile pair (2v, 2v+1) = (half0, half1) signs, nonzero only in
    # out rows v*16..v*16+16 -- eight super-tiles accumulate into disjoint
    # 16-row stripes of one full [128, FD] PSUM tile (DoubleRow dst
    # partition base must be 0, so the out always spans all 128 rows).
    cm = np.zeros((128, 16, 128), np.float32)
    cv = np.zeros((128, 4), np.float32)
    for half in (0, 1):
        for kl in range(8):
            k = half * 8 + kl
            for g in range(G):
                m = kl * G + g
                for s in range(NF):
                    w1[s * G + g, half * 128 + m] = slotw[k, s]
                for v in range(8):
                    cm[m, v * 2 + half, v * 16 + g] = signs[k]
                cv[m, half] = const[k] - logcmax
                cv[m, 2 + half] = (const[k] - logcmax) * SCHRAUD_A1 + SCHRAUD_B1
    return w1.astype(bf16), cm.reshape(128, 16 * 128).astype(f8), cv, logcmax


def _build_bass():
    nc = bacc.Bacc("TRN2", target_bir_lowering=False, debug=False)
    f32 = mybir.dt.float32
    bf16 = mybir.dt.bfloat16
    f8 = mybir.dt.float8e4
    u8 = mybir.dt.uint8
    rhs_d = nc.dram_tensor("rhs", [NF * G, NST * FD], bf16, kind="ExternalInput")
    w1_d = nc.dram_tensor("w1", [NF * G, 2 * 128], bf16, kind="ExternalInput")
    cm_d = nc.dram_tensor("cm", [128, 16 * 128], f8, kind="ExternalInput")
    cv_d = nc.dram_tensor("cv", [128, 4], f32, kind="ExternalInput")
    acc_d = nc.dram_tensor("acc", [128, NBLK], f32, kind="ExternalOutput")

    assert sum(RHS_CHUNKS) == NST

    with tile.TileContext(nc) as tc:
        with (
            tc.tile_pool(name="const", bufs=1) as cpool,
            tc.tile_pool(name="rhs", bufs=RHS_BUFS) as rpool,
            tc.tile_pool(name="dens", bufs=DENS_BUFS) as dpool,
            tc.tile_pool(name="pe", bufs=E_BUFS, space="PSUM") as epool,
            tc.tile_pool(name="pg", bufs=G_BUFS, space="PSUM") as gpool,
        ):
            w1 = cpool.tile([NF * G, 2 * 128], bf16)
            cm = cpool.tile([128, 16, 128], f8)
            cv = cpool.tile([128, 4], f32)
            acc = cpool.tile([128, NBLK], f32)
            scr = cpool.tile([128, FD], bf16)  # square dummy out
            wsb = cpool.tile([128, FD], bf16)  # warm-up operand

            # rhs chunk 0 first: its transfer overlaps the remaining issue
            # chain and the program preamble.
            rhs_views = {}
            chunk_tiles = []
            lo = 0
            for ci, sz in enumerate(RHS_CHUNKS):
                hi = lo + sz
                rt = rpool.tile([NF * G, max(RHS_CHUNKS) * FD], bf16, tag="rhs")
                nc.sync.dma_start(rt[:, : sz * FD], rhs_d[:, lo * FD : hi * FD])
                for st in range(lo, hi):
                    rhs_views[st] = rt[:, (st - lo) * FD : (st - lo + 1) * FD]
                lo = hi
                chunk_tiles.append(rt)
                if ci == 0:
                    # w1+cv are needed by the first mm1/exp; cm only at the
                    # first mm2 (PIPE_D supertiles later), so it can wait
                    # until after the second rhs chunk.
                    nc.sync.dma_start(w1[:], w1_d[:])
                    nc.sync.dma_start(cv[:], cv_d[:])
                elif ci == 1:
                    nc.sync.dma_start(cm[:, :, :], cm_d[:])

            # PE warm-up on memset tiles (no DMA dependency): heats the HAM
            # clock-gate during the preamble + first chunk's flight time.
            nc.vector.memset(wsb[:], 0.0)
            warm = gpool.tile([128, FD], f32, tag="g", name="warm")
            for _ in range(NWARM):
                nc.tensor.matmul(
                    warm[0:128, 0:WARM_FD],
                    wsb[:, 0:128],
                    wsb[:, 0:WARM_FD],
                    start=True,
                    stop=True,
                )

            dens_ring = [None] * NST
            g_ring = [None, None]

            def emit_front(st):
                rhs = rhs_views[st]
                e = epool.tile([128, 2 * FD], f32, tag="e")
                d = dpool.tile([128, 2, FD], f8, tag="dens")
                nc.tensor.matmul(
                    e[:, 0:FD], w1[:, 0:128], rhs, start=True, stop=True
                )
                nc.tensor.matmul(
                    e[:, FD : 2 * FD], w1[:, 128:256], rhs, start=True, stop=True
                )
                for h in (0, 1):
                    eng = EXP_PAT[(st * 2 + h) % len(EXP_PAT)]
                    eh = e[:, h * FD : (h + 1) * FD]
                    dh = d[:, h, :]
                    if eng == "A":
                        nc.scalar.activation(
                            dh,
                            eh,
                            mybir.ActivationFunctionType.Exp,
                            bias=cv[:, h : h + 1],
                        )
                    else:
                        nc.vector.tensor_scalar(
                            dh.bitcast(u8),
                            eh,
                            SCHRAUD_A1,
                            cv[:, 2 + h : 3 + h],
                            op0=mybir.AluOpType.mult,
                            op1=mybir.AluOpType.add,
                        )
                dens_ring[st] = d

            def emit_back(st):
                d = dens_ring[st]
                dens_ring[st] = None
                v = st % 8
                if v == 0:
                    g_ring[(st // 8) % 2] = gpool.tile(
                        [128, FD], f32, tag="g", name=f"g128_{st // 8}"
                    )
                g128 = g_ring[(st // 8) % 2]
                nc.tensor.matmul(
                    g128[:, :],
                    cm[:, 2 * v : 2 * v + 2, :],
                    d[:, :, :],
                    start=(v == 0),
                    stop=(v == 7),
                    perf_mode=mybir.MatmulPerfMode.DoubleRow,
                    skip_group_check=True,
                )
                if v == 7:
                    blk = st // 8
                    nc.scalar.activation(
                        scr[:],
                        g128[:],
                        mybir.ActivationFunctionType.Square,
                        accum_out=acc[:, blk : blk + 1],
                    )

            D = PIPE_D
            for t in range(NST + D):
                if t < NST:
                    emit_front(t)
                if t >= D:
                    emit_back(t - D)

            nc.sync.dma_start(acc_d[:], acc[:])

    nc.compile()
    return nc


def _get_bass():
    if "nc" not in _CACHE:
        _CACHE["nc"] = _build_bass()
    return _CACHE["nc"]


def kernel(X, means, chols, weights, it=None, **_unused):
    X = np.ascontiguousarray(np.asarray(X, np.float32))
    assert X.shape == (N_SAMPLES, 2), X.shape

    A, signs, ctr = _cluster_params(means, chols, weights)
    z = _z_term(means, chols, weights)
    w1, cm, cv, logcmax = _build_weights(A, signs)
    rhs = _build_rhs(X, ctr)

    nc = _get_bass()
    in_maps = [
        {"rhs": rhs[c], "w1": w1, "cm": cm, "cv": cv} for c in range(N_CORES)
    ]
    res = run_bass_kernel_spmd(nc, in_maps, core_ids=list(range(N_CORES)))

    total = 0.0
    for r in res.results:
        total += float(r["acc"].astype(np.float64).sum())
    # dens were scaled by exp(-logcmax) on device; undo on the squared sum.
    out = -(np.log(total) + 2.0 * logcmax - np.log(z)) / N_SAMPLES
    return np.float32(out)


if __name__ == "__main__":
    rng = np.random.default_rng(0)
    X = rng.standard_normal((N_SAMPLES, 2), dtype=np.float32)
    scale = 2.0 * (1.0 + rng.standard_normal((K, 1, 1), dtype=np.float32))
    chols = scale * np.ones((2, 2), np.float32)[None] + 0.5 * np.eye(2, dtype=np.float32)[None]
    means = rng.standard_normal((K, 2), dtype=np.float32)
    weights = rng.standard_normal(K, dtype=np.float32)
    print(kernel(X, means, chols, weights, 1))
